# revision 21
# baseline (speedup 1.0000x reference)
"""Cross-attention Trainium2 kernel (Bass/Tile), 8-core SPMD.

Sharding: 8 cores = 2 (batch) x 4 (head groups of 3 heads).
Each core computes, for its (b, g):
    q^T = Wq_g @ x_b^T          [192, 2048]  (+bq)
    k^T = Wk_g @ y_b^T          [192, 2048]  (+bk)
    v   = y_b @ Wv_g^T          [2048, 192]
    per head: S^T = k_h q_h^T   [2048(m), 2048(l)] tiles in PSUM
              P^T = exp(S^T/8)  (softmax numerator, bf16)
              O^T = v_h^T P^T   (PSUM accumulated over m)
              den = 1^T P^T, O_n^T = O^T * (1/den)  (broadcast via PE)
    partial^T = Wp_g^T^T @ O_n^T  [768, 2048]  -> DRAM fp32
Host: out[b] = sum_g partial_g^T.T + Wp @ bv + bp.

All device matmuls in bf16 (fp32 PSUM accumulation); exp on ACT engine.
"""

import os
import sys
from contextlib import ExitStack

import numpy as np

for _p in ("/opt/trn_rl_repo", "/root/.axon_site/_ro/trn_rl_repo"):
    if os.path.isdir(_p) and _p not in sys.path:
        sys.path.insert(0, _p)

try:  # make trace=True harmless when the env lacks the NTFF hook module
    import antenv.axon_hooks  # noqa: F401
except Exception:
    import types

    _stub = types.ModuleType("antenv.axon_hooks")
    _stub.get_axon_ntff_profile_hook = lambda: None
    _stub.set_axon_ntff_profile_hook = lambda hook: None
    sys.modules["antenv.axon_hooks"] = _stub

import concourse.bass as bass
import concourse.tile as tile
from concourse import bacc as bacc_mod
from concourse import mybir
from concourse.bass_utils import run_bass_kernel_spmd
from ml_dtypes import bfloat16

F32 = mybir.dt.float32
F32R = mybir.dt.float32r
BF16 = mybir.dt.bfloat16
EXP = mybir.ActivationFunctionType.Exp

B = 2
L = 2048          # query length (also key length)
D = 768
HD = 64           # head dim
HPC = 3           # heads per core
GW = HPC * HD     # 192: head-group width
KT = D // 128     # 6 contraction tiles for V projection
KTA = KT + 1      # 7 tiles for Q/K: 7th carries the bias row (exact bias fold)
DA = KTA * 128    # 896: augmented contraction depth
NLC = L // 512    # 4 l-chunks
NM = L // 128     # 16 m-tiles
SCALE = 1.0 / 8.0  # hd ** -0.5


def _build_program():
    nc = bacc_mod.Bacc()

    xT = nc.dram_tensor("xT", [DA, L], BF16, kind="ExternalInput")[:, :]
    yT = nc.dram_tensor("yT", [DA, L], BF16, kind="ExternalInput")[:, :]
    wqT = nc.dram_tensor("wqT", [DA, GW], BF16, kind="ExternalInput")[:, :]
    wkT = nc.dram_tensor("wkT", [DA, GW], BF16, kind="ExternalInput")[:, :]
    wvT = nc.dram_tensor("wvT", [D, GW], BF16, kind="ExternalInput")[:, :]
    wpT = nc.dram_tensor("wpT", [GW, D], BF16, kind="ExternalInput")[:, :]
    pT = nc.dram_tensor("pT", [D, L], F32, kind="ExternalOutput")[:, :]

    with tile.TileContext(nc) as tc, ExitStack() as ctx:
        persist = ctx.enter_context(tc.tile_pool(name="persist", bufs=1))
        ppool = ctx.enter_context(tc.tile_pool(name="ppool", bufs=2, space="PSUM"))
        spool = ctx.enter_context(tc.tile_pool(name="spool", bufs=3, space="PSUM"))
        ptpool = ctx.enter_context(tc.tile_pool(name="ptpool", bufs=6))
        accpool = ctx.enter_context(tc.tile_pool(name="accpool", bufs=2))
        rpool = ctx.enter_context(tc.tile_pool(name="rpool", bufs=2))
        bcpool = ctx.enter_context(tc.tile_pool(name="bcpool", bufs=2))

        # ---------------- load inputs (chunked across DMA queues) ----------
        xT_sb = persist.tile([128, KTA, L], BF16, tag="xT")
        yT_sb = persist.tile([128, KTA, L], BF16, tag="yT")
        wq_sb = persist.tile([128, KTA, GW], BF16, tag="wq")
        wk_sb = persist.tile([128, KTA, GW], BF16, tag="wk")
        wv_sb = persist.tile([128, KT, GW], BF16, tag="wv")
        wp_a = persist.tile([128, D], BF16, tag="wpa")
        wp_b = persist.tile([64, D], BF16, tag="wpb")

        xT_r = xT.rearrange("(kt p) l -> p kt l", p=128)
        yT_r = yT.rearrange("(kt p) l -> p kt l", p=128)
        # y l-half0 + weights + x l-half0 first: the first K/V/Q projection
        # chunks and slot-B attention only need those.
        h0 = slice(0, L // 2)
        h1 = slice(L // 2, L)
        for kt in range(KTA):
            nc.sync.dma_start(out=yT_sb[:, kt, h0], in_=yT_r[:, kt, h0])
        nc.sync.dma_start(out=wk_sb, in_=wkT.rearrange("(kt p) g -> p kt g", p=128))
        nc.sync.dma_start(out=wv_sb, in_=wvT.rearrange("(kt p) g -> p kt g", p=128))
        nc.sync.dma_start(out=wq_sb, in_=wqT.rearrange("(kt p) g -> p kt g", p=128))
        for kt in range(KTA):
            nc.sync.dma_start(out=xT_sb[:, kt, h0], in_=xT_r[:, kt, h0])
        for kt in range(KTA):
            nc.sync.dma_start(out=yT_sb[:, kt, h1], in_=yT_r[:, kt, h1])
        for kt in range(KTA):
            nc.sync.dma_start(out=xT_sb[:, kt, h1], in_=xT_r[:, kt, h1])
        nc.sync.dma_start(out=wp_a, in_=wpT[0:128, :])
        nc.sync.dma_start(out=wp_b, in_=wpT[128:GW, :])

        ones_col = persist.tile([128, 1], BF16, tag="onesc")
        nc.vector.memset(ones_col, 1.0)

        # persistent activation tensors
        qT_p = persist.tile([128, L], BF16, tag="qTp")   # heads 0,1 stacked
        qT_2 = persist.tile([128, L], BF16, tag="qT2")   # head 2, dup halves
        kT_p = persist.tile([128, L], BF16, tag="kTp")
        kT_2 = persist.tile([128, L], BF16, tag="kT2")   # head 2, dup halves
        v_sb = persist.tile([128, NM, GW], BF16, tag="v")
        on_p = persist.tile([128, L], BF16, tag="onp")   # normalized O^T heads 0,1
        on_2 = persist.tile([64, L], BF16, tag="on2")    # head 2

        # ---------------- emission helpers ----------------
        def k_chunk(lc):
            sl = slice(lc * 512, (lc + 1) * 512)
            ps = ppool.tile([128, 512], F32, tag="ps")
            for kt in range(KTA):
                nc.tensor.matmul(ps, wk_sb[:, kt, 0:128], yT_sb[:, kt, sl],
                                 start=(kt == 0), stop=(kt == KTA - 1))
            nc.vector.tensor_copy(kT_p[:, sl], ps)
            ps2 = ppool.tile([64, 512], F32, tag="ps")
            for kt in range(KTA):
                nc.tensor.matmul(ps2, wk_sb[:, kt, 128:GW], yT_sb[:, kt, sl],
                                 start=(kt == 0), stop=(kt == KTA - 1))
            nc.vector.tensor_copy(kT_2[0:64, sl], ps2)
            nc.vector.tensor_copy(kT_2[64:128, sl], ps2)

        def v_chunk(m):
            ms = slice(m * 128, (m + 1) * 128)
            ps = ppool.tile([128, GW], F32, tag="ps")
            for kt in range(KT):
                nc.tensor.matmul(ps, yT_sb[:, kt, ms], wv_sb[:, kt, :],
                                 start=(kt == 0), stop=(kt == KT - 1))
            nc.vector.tensor_copy(v_sb[:, m, :], ps)

        def q_chunk(lc, pair):
            sl = slice(lc * 512, (lc + 1) * 512)
            if pair:
                ps = ppool.tile([128, 512], F32, tag="ps")
                for kt in range(KTA):
                    nc.tensor.matmul(ps, wq_sb[:, kt, 0:128], xT_sb[:, kt, sl],
                                     start=(kt == 0), stop=(kt == KTA - 1))
                nc.vector.tensor_copy(qT_p[:, sl], ps)
            else:
                ps2 = ppool.tile([64, 512], F32, tag="ps")
                for kt in range(KTA):
                    nc.tensor.matmul(ps2, wq_sb[:, kt, 128:GW], xT_sb[:, kt, sl],
                                     start=(kt == 0), stop=(kt == KTA - 1))
                nc.vector.tensor_copy(qT_2[0:64, sl], ps2)
                nc.vector.tensor_copy(qT_2[64:128, sl], ps2)

        def _acc_add(m, acc, pt):
            c = m % 2  # two interleaved chains halve the serial dep depth
            if m < 2:
                nc.vector.tensor_copy(acc[:, c, :], pt)
            else:
                nc.vector.tensor_add(acc[:, c, :], acc[:, c, :], pt)

        def norm_one(o_ap, acc, asl, dst_ap):
            """normalize one head/l-chunk: dst = o / den; den sums the DVE
            and GpSimd accumulator chains."""
            den = ppool.tile([1, 512], F32, tag="ps")
            nc.tensor.matmul(den, ones_col, acc[:, 0, asl], start=True, stop=False)
            nc.tensor.matmul(den, ones_col, acc[:, 1, asl], start=False, stop=True)
            recip = rpool.tile([1, 512], F32, tag="recip")
            nc.vector.reciprocal_approx_fast(out=recip, in_=den)
            bc = bcpool.tile([64, 512], F32, tag="bc")
            nc.gpsimd.partition_broadcast(bc, recip)
            nc.vector.tensor_mul(dst_ap, o_ap, bc)

        def slot_a_m(m, sl, s_ps, o_ps, acc, first, last):
            ms = slice(m * 128, (m + 1) * 128)
            nc.tensor.matmul(s_ps[:, 0:512], kT_p[0:64, ms], qT_p[0:64, sl],
                             tile_position=(0, 0), start=True, stop=True)
            nc.tensor.matmul(s_ps[:, 512:1024], kT_p[64:128, ms], qT_p[64:128, sl],
                             tile_position=(64, 0), start=True, stop=True)
            pt = ptpool.tile([128, 1024], BF16, tag="pt")
            nc.scalar.activation(pt, s_ps, EXP, scale=SCALE)
            nc.tensor.matmul(o_ps[0:64, :], v_sb[:, m, 0:64], pt[:, 0:512],
                             tile_position=(0, 0), start=first, stop=last)
            nc.tensor.matmul(o_ps[64:128, :], v_sb[:, m, 64:128], pt[:, 512:1024],
                             tile_position=(0, 64), start=first, stop=last)
            _acc_add(m, acc, pt)

        def slot_b_m(m, sl0, sl1, s_ps, o_ps, acc, first, last):
            ms = slice(m * 128, (m + 1) * 128)
            nc.tensor.matmul(s_ps[:, 0:512], kT_2[0:64, ms], qT_2[0:64, sl0],
                             tile_position=(0, 0), start=True, stop=True)
            nc.tensor.matmul(s_ps[:, 512:1024], kT_2[64:128, ms], qT_2[64:128, sl1],
                             tile_position=(64, 0), start=True, stop=True)
            pt = ptpool.tile([128, 1024], BF16, tag="pt")
            nc.scalar.activation(pt, s_ps, EXP, scale=SCALE)
            nc.tensor.matmul(o_ps[0:64, :], v_sb[:, m, 128:GW], pt[:, 0:512],
                             tile_position=(0, 0), start=first, stop=last)
            nc.tensor.matmul(o_ps[64:128, :], v_sb[:, m, 128:GW], pt[:, 512:1024],
                             tile_position=(0, 64), start=first, stop=last)
            _acc_add(m, acc, pt)

        def p_proj(lc):
            sl = slice(lc * 512, (lc + 1) * 512)
            for o in range(D // 128):
                osl = slice(o * 128, (o + 1) * 128)
                ps = ppool.tile([128, 512], F32, tag="ps")
                nc.tensor.matmul(ps, wp_a[:, osl], on_p[:, sl], start=True, stop=False)
                nc.tensor.matmul(ps, wp_b[:, osl], on_2[:, sl], start=False, stop=True)
                po = ptpool.tile([128, 512], F32, tag="po")
                nc.vector.tensor_copy(po, ps)
                nc.sync.dma_start(out=pT[osl, sl], in_=po)

        # ---------------- software-pipelined emission ----------------
        # head-2 slot first, with K/V projection chunks threaded between its
        # m-blocks so ACT starts ~10us in and PE never idles on a phase edge.
        k_chunk(0)
        for m in range(4):
            v_chunk(m)
        q_chunk(0, pair=False)
        q_chunk(1, pair=False)

        # slot B, lc-pair 0 (covers l chunks 0,1)
        o_ps_b0 = ppool.tile([128, 512], F32, tag="ps")
        acc_b0 = accpool.tile([128, 2, 1024], BF16, tag="acc")
        sl0, sl1 = slice(0, 512), slice(512, 1024)
        for m in range(NM):
            s_ps = spool.tile([128, 1024], F32, tag="s")
            slot_b_m(m, sl0, sl1, s_ps, o_ps_b0, acc_b0, m == 0, m == NM - 1)
            if m == 3:
                k_chunk(1)
                for mm in range(4, 8):
                    v_chunk(mm)
            elif m == 7:
                k_chunk(2)
                for mm in range(8, 12):
                    v_chunk(mm)
            elif m == 11:
                k_chunk(3)
                for mm in range(12, 16):
                    v_chunk(mm)
        norm_one(o_ps_b0[0:64, :], acc_b0, slice(0, 512), on_2[:, sl0])
        norm_one(o_ps_b0[64:128, :], acc_b0, slice(512, 1024), on_2[:, sl1])
        q_chunk(2, pair=False)
        q_chunk(3, pair=False)

        # slot B, lc-pair 1 (l chunks 2,3)
        o_ps_b1 = ppool.tile([128, 512], F32, tag="ps")
        acc_b1 = accpool.tile([128, 2, 1024], BF16, tag="acc")
        sl2, sl3 = slice(1024, 1536), slice(1536, 2048)
        for m in range(NM):
            s_ps = spool.tile([128, 1024], F32, tag="s")
            slot_b_m(m, sl2, sl3, s_ps, o_ps_b1, acc_b1, m == 0, m == NM - 1)
            if m == 3:
                q_chunk(0, pair=True)
            elif m == 7:
                q_chunk(1, pair=True)
            elif m == 11:
                q_chunk(2, pair=True)
        norm_one(o_ps_b1[0:64, :], acc_b1, slice(0, 512), on_2[:, sl2])
        norm_one(o_ps_b1[64:128, :], acc_b1, slice(512, 1024), on_2[:, sl3])

        # slot A per l-chunk, with q-proj of the next chunk and p-proj of the
        # previous chunk threaded into the m-loop
        for lc in range(NLC):
            sl = slice(lc * 512, (lc + 1) * 512)
            o_ps = ppool.tile([128, 512], F32, tag="ps")
            acc = accpool.tile([128, 2, 1024], BF16, tag="acc")
            for m in range(NM):
                s_ps = spool.tile([128, 1024], F32, tag="s")
                slot_a_m(m, sl, s_ps, o_ps, acc, m == 0, m == NM - 1)
                if m == 7 and lc == 0:
                    q_chunk(3, pair=True)
                elif m == 7 and lc > 0:
                    p_proj(lc - 1)
            norm_one(o_ps[0:64, :], acc, slice(0, 512), on_p[0:64, sl])
            norm_one(o_ps[64:128, :], acc, slice(512, 1024), on_p[64:128, sl])
        p_proj(NLC - 1)

    nc.finalize()
    return nc


_NC = None


def _get_nc():
    global _NC
    if _NC is None:
        _NC = _build_program()
    return _NC


def _aug_act(a):
    """[L, D] activations -> [DA, L]: transpose, append ones row + zero pad."""
    out = np.zeros((DA, L), dtype=bfloat16)
    out[:D] = a.T.astype(bfloat16)
    out[D] = 1.0
    return out


def _aug_w(w_rows, b_rows):
    """[GW, D] weight rows + [GW] bias -> [DA, GW] lhsT with bias row."""
    out = np.zeros((DA, GW), dtype=bfloat16)
    out[:D] = w_rows.T.astype(bfloat16)
    out[D] = b_rows.astype(bfloat16)
    return out


def _make_in_maps(x, y, Wq, bq, Wk, bk, Wv, bv, Wp, bp):
    in_maps = []
    xTs = [_aug_act(x[b]) for b in range(B)]
    yTs = [_aug_act(y[b]) for b in range(B)]
    for core in range(8):
        b, g = divmod(core, 4)
        rows = slice(g * GW, (g + 1) * GW)
        in_maps.append({
            "xT": xTs[b],
            "yT": yTs[b],
            "wqT": _aug_w(Wq[rows], bq[rows]),
            "wkT": _aug_w(Wk[rows], bk[rows]),
            "wvT": np.ascontiguousarray(Wv[rows].T).astype(bfloat16),
            "wpT": np.ascontiguousarray(Wp[:, rows].T).astype(bfloat16),
        })
    return in_maps


def _combine(results, Wv_b, Wp, bp, bv):
    out = np.zeros((B, L, D), dtype=np.float32)
    for core in range(8):
        b = core // 4
        out[b] += results[core]["pT"].T
    out += (Wp @ bv + bp)[None, None, :]
    return out


def run(inputs, trace=False, trace_cores=None, **kwargs):
    nc = _get_nc()
    in_maps = _make_in_maps(**inputs)
    res = run_bass_kernel_spmd(
        nc, in_maps, core_ids=list(range(8)), trace=trace,
        trace_cores=trace_cores, **kwargs)
    out = _combine(res.results, inputs["Wv"], inputs["Wp"],
                   inputs["bp"], inputs["bv"])
    return out, res


def kernel(**inputs):
    inputs = {k: np.asarray(v) for k, v in inputs.items()}
    out, _ = run(inputs, trace=False)
    return out
